# revision 25
# baseline (speedup 1.0000x reference)
"""Trainium2 Bass kernel for nn_EncodingLayer (dense transformer encoder layer).

Reference computation (B=2, S=2048, H=128, NH=8):
    Q/K/V = per-head full-dim projections of x, scores = QK^T/sqrt(H),
    A = softmax(scores), o = A@V, concat heads, y = o@Wo+bo,
    y = LN1(y), f = relu(relu(y@W1+b1)@W2+b2), out = LN2(y+f).

Sharding: data-parallel over query rows. Core c (of 8) owns batch b=c//4 and
query rows q0=(c%4)*512 .. q0+512 of that batch. Each core computes K for
its full batch (4x replicated compute, tiny) and the full epilogue for its
512 rows. No collectives; host concatenates the 8 [512,128] slices.

Key structure per core:
  Algebra: V and Wo are never applied separately. o@Wo is reassociated as
      y_h = (A_h X Wv_h + bv_h) Wo_h = (A_h X) (Wv_h Wo_h) + bv_h Wo_h,
  with Wvo_h = Wv_h@Wo_h and sum_h bv_h@Wo_h + bo folded on the HOST. So the
  attention t-loop contracts P^T against natural-layout bf16 x chunks (same
  PE cost as A@V), and ONE per-head matmul accumulates straight into y^T.
  This deletes the V projection, its 16 PSUM drains, the per-head Wo matmul,
  and enough PSUM pressure that K/Q projections share the scores PSUM ring —
  letting them interleave per-head with attention so head 0 starts as soon
  as x and the first weight columns land (instead of after a serial
  projection phase).

  Phase 0 (load): weights arrive as TWO host-packed [128, X] blobs (one DMA
    trigger costs ~950ns on the issuing engine); x in two chunks. Triggers
    split across the two HWDGE engines (sync + scalar). A fixed ~7us engine
    boot preamble precedes everything; ACT activation tables are preloaded
    during it so no table load lands mid-epilogue.
  Per head: K^T/Q^T matmuls (PSUM ring shared with scores, DVE drains), then
    a pure t-loop: scores^T chunks on PE (bf16), P^T = exp on ACT straight
    out of PSUM (bf16), denominator via ones-vector matmul (sum over the t
    partition dim), m = (AX)^T accumulated on PE. Softmax without
    max-subtraction is numerically exact at this problem scale (|scores|<.5).
    Finalize (DVE): mnorm = m * approx-recip(denom); y^T += Wvo_h^T mnorm.
  Epilogue (fully transposed, two column halves): LN1/FFN/LN2 in [feature,
    seq] layout; LN stats over the feature dim (= partition dim) via
    ones-matmul column sums — no transposes until the final output. The
    gain*x+beta step rides ACT's per-partition scale/bias; second-half
    elementwise tail goes to GpSimd so the halves overlap. Output DMAs are
    split across both HWDGE engines (a single queue writes only ~22GB/s).
"""

import math
import numpy as np
from contextlib import ExitStack

import concourse.bass as bass
import concourse.bacc as bacc
import concourse.mybir as mybir
import concourse.tile as tile
from concourse.bass_utils import run_bass_kernel_spmd
from concourse.masks import make_identity

B, S, H, NH = 2, 2048, 128, 8
F = 2 * H                      # FFN hidden dim (256)
NCORES = 8
SQ = (B * S) // NCORES         # 512 query rows per core
TC = S // 128                  # 16 key chunks of 128
CH = SQ // 2                   # epilogue column half
LN_EPS = 1e-5
FP32 = mybir.dt.float32
FP32R = mybir.dt.float32r
BF16 = mybir.dt.bfloat16
AF = mybir.ActivationFunctionType
ALU = mybir.AluOpType

# wall_a column offsets (fp32 cols): wk | wq | bq | bk
WA_WK, WA_WQ = 0, 1024
WA_COLS = 2048
# wall_b: wvo | w1 | w2 | bo2 | b1 | b2 | g1 | be1 | g2 | be2
WB_WVO, WB_W1, WB_W2 = 0, 1024, 1280
WB_BO, WB_B1, WB_B2 = 1536, 1537, 1539
WB_G1, WB_BE1, WB_G2, WB_BE2 = 1540, 1541, 1542, 1543
WB_BQ, WB_BK = 1544, 1552
WB_COLS = 1560


def _r(ap):
    return ap.bitcast(FP32R)


def build_module():
    nc = bacc.Bacc(None)

    xb_d = nc.declare_dram_parameter("xb", [S, H], FP32, isOutput=False)
    xq_d = nc.declare_dram_parameter("xq", [SQ, H], FP32, isOutput=False)
    wa_d = nc.declare_dram_parameter("wall_a", [128, WA_COLS], FP32R, isOutput=False)
    wb_d = nc.declare_dram_parameter("wall_b", [128, WB_COLS], FP32R, isOutput=False)
    out_d = nc.declare_dram_parameter("out", [SQ, H], FP32, isOutput=True)

    with tile.TileContext(nc) as tc, ExitStack() as ctx:
        singles = ctx.enter_context(tc.tile_pool(name="singles", bufs=1))

        # ---- constants ----
        ident = singles.tile([128, 128], FP32)
        make_identity(nc, ident[:])
        ones128 = singles.tile([128, 128], FP32)  # all-ones lhsT: partition sums
        nc.vector.memset(ones128[:], 1.0)
        ones_bf = singles.tile([128, 128], BF16)  # bf16 twin for bf16 matmuls
        nc.vector.memset(ones_bf[:], 1.0)
        eps_t = singles.tile([128, 1], FP32)
        nc.vector.memset(eps_t[:], LN_EPS)
        # Preload ACT tables (Sqrt lives in a different table than Exp etc.;
        # without this a 1.3us ACT_TABLE_LOAD lands mid-LayerNorm).
        scr1 = singles.tile([128, 1], FP32)
        nc.scalar.activation(out=scr1[:], in_=eps_t[:], func=AF.Sqrt, bias=eps_t[:])
        nc.scalar.activation(out=scr1[:], in_=eps_t[:], func=AF.Square)
        nc.scalar.activation(out=scr1[:], in_=eps_t[:], func=AF.Exp)

        # ---- DMAs: x halves + packed weight blobs, split across engines ----
        xb_sb = singles.tile([128, TC, H], FP32)  # (s%128, sc, d)
        xb_r = xb_d[:].rearrange("(sc p) d -> p sc d", p=128)
        wa_sb = singles.tile([128, WA_COLS], FP32R)
        nc.sync.dma_start(out=xb_sb[:, 0:8, :], in_=xb_r[:, 0:8, :])
        nc.sync.dma_start(out=wa_sb[:], in_=wa_d[:])
        nc.sync.dma_start(out=xb_sb[:, 8:16, :], in_=xb_r[:, 8:16, :])
        xq_sb = singles.tile([128, SQ // 128, H], FP32)
        nc.scalar.dma_start(out=xq_sb[:], in_=xq_d[:].rearrange("(sc p) d -> p sc d", p=128))
        wb_sb = singles.tile([128, WB_COLS], FP32R)
        nc.scalar.dma_start(out=wb_sb[:], in_=wb_d[:])

        def wa(c0, n):          # fp32r view (matmul operands)
            return wa_sb[:, c0:c0 + n]

        def wb(c0, n):
            return wb_sb[:, c0:c0 + n]

        def waf(c0, n):         # plain-fp32 view (DVE/ACT/GpSimd operands)
            return wa_sb[:, c0:c0 + n].bitcast(FP32)

        def wbf(c0, n):
            return wb_sb[:, c0:c0 + n].bitcast(FP32)

        xT = singles.tile([H, S], FP32)
        xqT = singles.tile([H, SQ], FP32)
        xb_bf = singles.tile([128, TC, H], BF16)  # natural x, bf16 (AX lhsT)

        # PE matmuls (fused LDWEIGHTS) can carry only ONE semaphore wait in
        # codegen. Dummy transposes/matmuls make PE observe one DMA/engine
        # semaphore so no later matmul needs to wait on two at once; _zd()
        # writes a [1,1] dummy into a new PSUM pool's first tile so the
        # pool-transition (released-zone) dependency is absorbed there
        # instead of landing on a real matmul that also has a data wait.
        def _zd(tile_ap):
            nc.tensor.matmul(tile_ap[0:1, 0:1], ident[:, 0:1], ident[:, 0:1],
                             start=True, stop=True)

        # ---- transposes: xT=[d, S], xqT=[d, SQ]; bf16 cast of natural x ----
        with tc.tile_pool(name="tp_ps", bufs=2, space="PSUM") as tp_ps:
            pt0 = tp_ps.tile([128, 128], FP32, tag="abs")
            nc.tensor.transpose(pt0[:], ident[:], ident[:])          # observe ident
            nc.tensor.transpose(pt0[:], xb_sb[:, 0, :], ident[:])    # observe xb half 0
            for sc in range(TC):
                pt = tp_ps.tile([128, 128], FP32, tag="tp")
                if sc == 0:
                    _zd(pt)
                nc.tensor.transpose(pt[:], xb_sb[:, sc, :], ident[:])
                nc.vector.tensor_copy(out=_r(xT[:, sc * 128:(sc + 1) * 128]), in_=pt[:])
                if sc % 4 == 3:
                    nc.vector.tensor_copy(out=xb_bf[:, sc - 3:sc + 1, :],
                                          in_=xb_sb[:, sc - 3:sc + 1, :])
            for sc in range(SQ // 128):
                pt = tp_ps.tile([128, 128], FP32, tag="tp")
                nc.tensor.transpose(pt[:], xq_sb[:, sc, :], ident[:])
                nc.vector.tensor_copy(out=_r(xqT[:, sc * 128:(sc + 1) * 128]), in_=pt[:])
            # observe the wall_a DMA before the projection matmuls
            nc.tensor.transpose(pt0[:], wa_sb[:, 0:128].bitcast(FP32), ident[:])

        kt_all = singles.tile([H, NH, S], BF16)       # (e, h, t)
        qt_all = singles.tile([H, NH, SQ], BF16)      # (e, h, s)

        # ---- attention: per head, K/Q projection (PSUM ring shared with
        # scores) then a pure t-loop ----
        pt_pool = ctx.enter_context(tc.tile_pool(name="pt", bufs=5))
        ot_pool = ctx.enter_context(tc.tile_pool(name="ot", bufs=2))

        yT_sb = singles.tile([H, SQ], FP32)  # attention block output (pre-LN), [j, s]

        with (
            tc.tile_pool(name="s_ps", bufs=3, space="PSUM") as s_ps,
            tc.tile_pool(name="kq_ps", bufs=2, space="PSUM") as kq_ps,
            tc.tile_pool(name="m_ps", bufs=1, space="PSUM") as m_ps,
            tc.tile_pool(name="d_ps", bufs=1, space="PSUM") as d_ps,
            tc.tile_pool(name="y_ps", bufs=1, space="PSUM") as y_ps,
        ):
            y_acc = y_ps.tile([H, SQ], FP32)
            _zd(y_acc)
            # Absorb wall_b and the DVE watermark of the transpose-phase
            # drains (xqT copies are DVE's last ops there) once up front.
            gp_abs = s_ps.tile([128, 512], FP32, tag="s")
            _zd(gp_abs)
            xq_view = xqT[0:1, SQ - 1:SQ]
            nc.tensor.matmul(gp_abs[0:1, 0:1], xq_view, xq_view, start=True, stop=True)
            wb_view = wb_sb[0:1, 0:1].bitcast(FP32)
            nc.tensor.matmul(gp_abs[0:1, 1:2], wb_view, wb_view, start=True, stop=True)

            def _proj(h):
                # Q first: its DVE drain lands right after the previous
                # head's finalize, so qt is ready before the first scores
                # matmul. K drains go to ACT: the scores matmuls then carry
                # a single ACT wait (max of kt-drain and sp-ring sems).
                qp = kq_ps.tile([128, 512], FP32, tag="kq")
                nc.tensor.matmul(qp[:], wa(WA_WQ + h * 128, 128), _r(xqT[:]),
                                 start=True, stop=True)
                nc.vector.tensor_scalar(
                    out=qt_all[:, h, :], in0=qp[:],
                    scalar1=wbf(WB_BQ + h, 1), scalar2=1.0 / math.sqrt(H),
                    op0=ALU.add, op1=ALU.mult,
                )
                for i in range(4):
                    kp = kq_ps.tile([128, 512], FP32, tag="kq")
                    nc.tensor.matmul(
                        kp[:], wa(WA_WK + h * 128, 128),
                        _r(xT[:, i * 512:(i + 1) * 512]),
                        start=True, stop=True,
                    )
                    nc.vector.tensor_scalar_add(
                        out=kt_all[:, h, i * 512:(i + 1) * 512], in0=kp[:],
                        scalar1=wbf(WB_BK + h, 1),
                    )

            # mnorm = m * approx-recip(denom); bv is folded into bo2 on host.
            def _finalize_dve(m_p, d_p):
                rec_bc = ot_pool.tile([128, SQ], FP32, tag="rec")
                nc.vector.reciprocal_approx_fast(out=rec_bc[:], in_=d_p[:])
                mnorm = ot_pool.tile([H, SQ], FP32, tag="mn")
                nc.vector.tensor_mul(out=_r(mnorm[:]), in0=m_p[:], in1=rec_bc[:])
                return mnorm

            prev = None  # (h, mnorm)
            _proj(0)
            for h in range(NH):
                kt = kt_all[:, h, :]
                qt = qt_all[:, h, :]

                m_acc = m_ps.tile([H, SQ], FP32, tag="m")
                d_acc = d_ps.tile([128, SQ], FP32, tag="d")
                if h == 0:
                    _zd(m_acc)
                    _zd(d_acc)

                # absorb the DVE watermark (kt/qt drains of this head) with
                # a dummy write into d_acc -- the first denominator matmul's
                # start=True reset clobbers it, so it is free.
                qv = kt_all[0:1, h, S - 2:S].bitcast(FP32)[:, 0:1]
                nc.tensor.matmul(d_acc[0:1, 0:1], qv, qv, start=True, stop=True,
                                 skip_group_check=True)

                # Software-pipelined t-loop at 512-column granularity: emit
                # scores three chunks ahead so PE always has work queued
                # while ACT computes the exps, and so no allocation ever
                # waits on the previous head's last exp.
                def _sc(c):
                    sp = s_ps.tile([128, 512], FP32, tag="s")
                    nc.tensor.matmul(sp[:], kt[:, c * 128:(c + 1) * 128], qt,
                                     start=True, stop=True)
                    pt = pt_pool.tile([128, 512], BF16, tag="pt")
                    nc.scalar.activation(out=pt[:], in_=sp[:], func=AF.Exp)
                    return pt

                pts = {c: _sc(c) for c in range(3)}
                for c in range(TC):
                    if c + 3 < TC:
                        pts[c + 3] = _sc(c + 3)
                    if c == 8 and h + 1 < NH:
                        _proj(h + 1)
                    pt = pts.pop(c)
                    nc.tensor.matmul(d_acc[:], ones_bf[:], pt[:],
                                     start=(c == 0), stop=(c == TC - 1),
                                     skip_group_check=(c == 0))
                    nc.tensor.matmul(m_acc[:], xb_bf[:, c, :], pt[:],
                                     start=(c == 0), stop=(c == TC - 1))

                mnorm = _finalize_dve(m_acc, d_acc)
                if prev is not None:
                    nc.tensor.matmul(y_acc[:], wb(WB_WVO + prev[0] * 128, 128),
                                     _r(prev[1][:]), start=(prev[0] == 0), stop=False)
                prev = (h, mnorm)

            nc.tensor.matmul(y_acc[:], wb(WB_WVO + (NH - 1) * 128, 128),
                             _r(prev[1][:]), start=False, stop=True)
            for hf in range(2):
                sl = slice(hf * CH, (hf + 1) * CH)
                nc.vector.tensor_scalar_add(out=_r(yT_sb[:, sl]), in0=y_acc[:, sl],
                                            scalar1=wbf(WB_BO, 1))

        # ---- epilogue, fully transposed, two pipelined column halves ----
        epi = ctx.enter_context(tc.tile_pool(name="epi", bufs=1))

        def _ln_stats(in_ap, ps_pool, hf, zd=False):
            t = str(hf)
            ysq = epi.tile([H, CH], FP32, tag="sq" + t)
            nc.scalar.activation(out=_r(ysq[:]), in_=in_ap, func=AF.Square)
            s_ps = ps_pool.tile([128, 2 * CH], FP32, tag="s" + t)
            if zd:
                _zd(s_ps)
            sum_y = s_ps[:, 0:CH]
            sum_q = s_ps[:, CH:2 * CH]
            nc.tensor.matmul(sum_y, _r(ones128[:]), _r(in_ap), start=True, stop=True)
            nc.tensor.matmul(sum_q, _r(ones128[:]), _r(ysq[:]), start=True, stop=True)
            m_sb = epi.tile([128, CH], FP32, tag="m" + t)
            nc.vector.tensor_scalar_mul(out=m_sb[:], in0=sum_y, scalar1=1.0 / H)
            t1 = epi.tile([128, CH], FP32, tag="t1" + t)
            nc.vector.scalar_tensor_tensor(
                out=t1[:], in0=sum_y, scalar=1.0 / H, in1=m_sb[:],
                op0=ALU.mult, op1=ALU.mult,
            )
            nc.vector.scalar_tensor_tensor(
                out=t1[:], in0=sum_q, scalar=1.0 / H, in1=t1[:],
                op0=ALU.mult, op1=ALU.subtract,
            )
            std = epi.tile([128, CH], FP32, tag="std" + t)
            nc.scalar.activation(out=std[:], in_=t1[:], func=AF.Sqrt, bias=eps_t[:])
            rstd = epi.tile([128, CH], FP32, tag="rs" + t)
            nc.vector.reciprocal_approx_fast(out=rstd[:], in_=std[:])
            return m_sb, rstd

        def _ln_tail(out_ap, in_ap, m_sb, rstd, g_col, beta_col, hf):
            # (x-m)*rstd on DVE (half 0) / GpSimd (half 1); gain+shift rides
            # ACT's per-partition scale/bias.
            t = str(hf)
            eng = nc.gpsimd
            ctr = epi.tile([128, CH], FP32, tag="ctr" + t)
            eng.tensor_sub(out=ctr[:], in0=in_ap, in1=m_sb[:])
            eng.tensor_mul(out=ctr[:], in0=ctr[:], in1=rstd[:])
            nc.scalar.activation(out=out_ap, in_=ctr[:], func=AF.Identity,
                                 scale=g_col, bias=beta_col)

        y1T = singles.tile([H, SQ], FP32)   # LN1 output, [j, s]
        uT = singles.tile([H, 2, SQ], FP32)
        rT = singles.tile([H, SQ], FP32)
        outT = singles.tile([H, SQ], FP32)
        out_sb = singles.tile([128, SQ // 128, H], FP32)
        out_r = out_d[:].rearrange("(sc p) j -> p sc j", p=128)

        with (
            tc.tile_pool(name="st_ps", bufs=1, space="PSUM") as st_ps,
            tc.tile_pool(name="u_ps", bufs=2, space="PSUM") as u_ps,
            tc.tile_pool(name="e_ps", bufs=2, space="PSUM") as e_ps,
        ):
            mr = [None, None]
            for hf in range(2):
                sl = slice(hf * CH, (hf + 1) * CH)
                mr[hf] = _ln_stats(yT_sb[:, sl], st_ps, hf, zd=(hf == 0))
            for hf in range(2):
                sl = slice(hf * CH, (hf + 1) * CH)
                _ln_tail(_r(y1T[:, sl]), yT_sb[:, sl], *mr[hf],
                         wbf(WB_G1, 1), wbf(WB_BE1, 1), hf)

            for hf in range(2):
                sl = slice(hf * CH, (hf + 1) * CH)
                # u^T[f, s] = relu(W1^T y1 + b1), f in two 128-chunks
                up = u_ps.tile([128, 2 * CH], FP32, tag="u")
                if hf == 0:
                    _zd(up)
                for fc in range(2):
                    nc.tensor.matmul(up[:, fc * CH:(fc + 1) * CH], wb(WB_W1 + fc * 128, 128),
                                     _r(y1T[:, sl]), start=True, stop=True)
                    nc.scalar.activation(out=_r(uT[:, fc, sl]), in_=up[:, fc * CH:(fc + 1) * CH],
                                         func=AF.Relu, bias=wbf(WB_B1 + fc, 1))
                # z^T[j, s] = relu(W2^T u + b2)
                zp = u_ps.tile([H, CH], FP32, tag="z")
                for fc in range(2):
                    nc.tensor.matmul(zp[:], wb(WB_W2 + fc * 128, 128), _r(uT[:, fc, sl]),
                                     start=(fc == 0), stop=(fc == 1))
                # residual in transposed space: rT = y1T + relu(zp + b2)
                nc.scalar.activation(out=_r(rT[:, sl]), in_=zp[:], func=AF.Relu,
                                     bias=wbf(WB_B2, 1))
                nc.vector.tensor_add(out=_r(rT[:, sl]), in0=rT[:, sl], in1=y1T[:, sl])

            for hf in range(2):
                sl = slice(hf * CH, (hf + 1) * CH)
                mr[hf] = _ln_stats(rT[:, sl], st_ps, hf)
            for hf in range(2):
                sl = slice(hf * CH, (hf + 1) * CH)
                _ln_tail(outT[:, sl], rT[:, sl], *mr[hf],
                         wbf(WB_G2, 1), wbf(WB_BE2, 1), hf)

                # back to natural layout, per half; store each half as soon
                # as it is ready, on alternating HWDGE engines.
                for sc in range(hf * 2, hf * 2 + 2):
                    op = e_ps.tile([128, 128], FP32, tag="e")
                    if hf == 0 and sc == 0:
                        _zd(op)
                    nc.tensor.transpose(op[:], outT[:, sc * 128:(sc + 1) * 128], ident[:])
                    nc.vector.tensor_copy(out=out_sb[:, sc, :], in_=op[:])
                heng = nc.sync if hf == 0 else nc.scalar
                heng.dma_start(out=out_r[:, hf * 2:hf * 2 + 2, :],
                               in_=out_sb[:, hf * 2:hf * 2 + 2, :])

    nc.finalize()
    return nc


_CACHE: dict = {}


def _get_nc():
    if "nc" not in _CACHE:
        _CACHE["nc"] = build_module()
    return _CACHE["nc"]


def _pack_walls(i):
    f32 = lambda a: np.ascontiguousarray(np.asarray(a), dtype=np.float32)
    wall_a = np.concatenate([
        f32(i["Wk"]).transpose(1, 0, 2).reshape(H, NH * H),
        f32(i["Wq"]).transpose(1, 0, 2).reshape(H, NH * H),
    ], axis=1)
    wo = f32(i["Wo"]).reshape(NH, H, H)          # (h, e, j)
    wv = f32(i["Wv"])                            # (h, d, e)
    wvo = np.einsum("hde,hej->dhj", wv, wo).reshape(H, NH * H)
    bo2 = f32(i["bo"]) + np.einsum("he,hej->j", f32(i["bv"]), wo)
    wall_b = np.concatenate([
        wvo,
        f32(i["W1"]),
        f32(i["W2"]).reshape(2, H, H).transpose(1, 0, 2).reshape(H, 2 * H),
        bo2[:, None],
        f32(i["b1"]).reshape(2, H).T,
        f32(i["b2"])[:, None],
        f32(i["g1"])[:, None], f32(i["beta1"])[:, None],
        f32(i["g2"])[:, None], f32(i["beta2"])[:, None],
        f32(i["bq"]).T, f32(i["bk"]).T,
    ], axis=1)
    assert wall_a.shape == (128, WA_COLS) and wall_b.shape == (128, WB_COLS)
    return np.ascontiguousarray(wall_a), np.ascontiguousarray(wall_b)


def _in_maps(inputs):
    x = np.ascontiguousarray(np.asarray(inputs["x"]), dtype=np.float32)
    wall_a, wall_b = _pack_walls(inputs)
    maps = []
    for c in range(NCORES):
        b, qi = divmod(c, NCORES // B)
        q0 = qi * SQ
        maps.append({
            "xb": np.ascontiguousarray(x[b]),
            "xq": np.ascontiguousarray(x[b, q0:q0 + SQ]),
            "wall_a": wall_a, "wall_b": wall_b,
        })
    return maps


def run(inputs, **kwargs):
    nc = _get_nc()
    res = run_bass_kernel_spmd(nc, _in_maps(inputs), core_ids=list(range(NCORES)), **kwargs)
    parts = [res.results[c]["out"] for c in range(NCORES)]
    y = np.concatenate(parts, axis=0).reshape(B, S, H).astype(np.float32)
    return y, res


def kernel(**inputs) -> np.ndarray:
    y, _ = run(inputs)
    return y
